# revision 2
# baseline (speedup 1.0000x reference)
"""Trainium2 Bass kernel for nn_BaseBranch_6811818132502 (dense_cnn).

Strategy (v2 — pass-pair M=128 bf16):
 - Host-side (exact): fold the channel-permutation einsum into conv1
   weights, fold rot90/rot-back into spatially rotated 3x3 kernels, fold
   BN scale INTO the weights (gamma>0 so the channel/pass max commutes),
   keep per-layer per-channel bias separate, replace pad-20 odd passes
   with pad-4 (exact; receptive field 5).  Weights and activations are
   bf16 (1 cycle/row on PE, same as fp32r, but enables M=128 weight
   loads that hide behind matmuls).
 - Device-side: data-parallel over batch (1 image per core, 8 cores).
   Passes are processed in PAIRS sharing the 128 output partitions:
   * conv1: both passes of a pair consume the same rhs (x + x shifted
     one row on partitions 64:127), so a dense [128,128] lhsT computes
     64 out-channels for each pass at once: 6 matmuls/chunk for TWO
     passes (3 tap-pair streams + 3 kh=2 singles).
   * conv2/conv3: y1/y2 hold pass A channels on partitions 0:63 and
     pass B on 64:127; block-diagonal [128,128] lhsT does tap (kh,kw)
     for both passes in one stream: 9 matmuls/chunk for TWO passes.
   Total matmul streams drop from 12/layer per pass-pair (v1) to 6-9.
 - Evictions: ACT relu+bias (scale already in weights) to bf16 for
   conv1/conv2; conv3 goes straight from PSUM into a running fp32
   channel max on DVE (bias+relu deferred past the max, which is exact
   because bias is per-channel pass-invariant and relu/max commute).
 - Tail: per-channel conv3 bias add (ACT), PE-transpose 128x128 blocks,
   DVE max-reduce, relu+sigmoid+clip, DMA out.
"""
import sys
import os
import math

for _p in ("/opt/trn_rl_repo", "/root/.axon_site/_ro/trn_rl_repo"):
    if os.path.isdir(_p) and _p not in sys.path:
        sys.path.insert(0, _p)

import numpy as np
import ml_dtypes

import concourse.bass as bass
import concourse.mybir as mybir
import concourse.tile as tile
from concourse import bacc, masks
from concourse.bass_utils import run_bass_kernel_spmd
from contextlib import ExitStack

F32 = mybir.dt.float32
BF16 = mybir.dt.bfloat16

BN_EPS = 1e-5
C = 64            # channels per pass
H = W = 96        # map size
B = 8             # batch == n_cores
PAD = 4           # explicit pad for odd passes (exact; receptive field 5)
R = 4             # output rows per PSUM chunk

XO_S = H + 2 * PAD + 2      # 106: x + pad4 + conv1 halo 1
Y1_S = H + 2 * PAD          # 104: conv1 odd output domain
Y2_S = H + 4                # 100: conv2 output domain

# pass pairs; evens first so odd passes overwrite the zero borders last
PAIRS = [(0, 2), (4, 6), (1, 3), (5, 7)]
N_SLOTS = 6 + 9 + 9         # conv1 streams + conv2 taps + conv3 taps
W_COLS = N_SLOTS * 128

# geometry per (layer, parity): dilation, rhs base offset, out rows/cols,
# output write offset into the destination buffer
GEOM = {
    (0, 0): dict(d=1, off=4, oh=96,  ow=96,  woff=4),   # conv1 even
    (0, 1): dict(d=1, off=0, oh=104, ow=104, woff=0),   # conv1 odd
    (1, 0): dict(d=2, off=2, oh=96,  ow=96,  woff=2),   # conv2 even
    (1, 1): dict(d=2, off=0, oh=100, ow=100, woff=0),   # conv2 odd
    (2, 0): dict(d=2, off=0, oh=96,  ow=96,  woff=None),  # conv3 -> ACC
    (2, 1): dict(d=2, off=0, oh=96,  ow=96,  woff=None),
}

_PROGRAM_CACHE = {}
TRACE = False
LAST_EXEC_NS = None


def _build_program():
    nc = bacc.Bacc("TRN2", target_bir_lowering=False, debug=False, num_devices=B)
    x_in = nc.dram_tensor("x_in", [C, H, W], BF16, kind="ExternalInput")
    w_in = nc.dram_tensor("w_in", [4, 128, W_COLS], BF16, kind="ExternalInput")
    b_in = nc.dram_tensor("b_in", [128, 3], F32, kind="ExternalInput")
    o_out = nc.dram_tensor("o_out", [1, H * W], F32, kind="ExternalOutput")

    with ExitStack() as ctx:
        tc = ctx.enter_context(tile.TileContext(nc))
        bigpool = ctx.enter_context(tc.tile_pool(name="big", bufs=1))
        wpool = ctx.enter_context(tc.tile_pool(name="wts", bufs=2))
        psum = ctx.enter_context(tc.tile_pool(name="ps", bufs=6, space="PSUM"))
        tpsum = ctx.enter_context(tc.tile_pool(name="tps", bufs=2, space="PSUM"))

        xo = bigpool.tile([128, XO_S, XO_S], BF16)
        y1 = bigpool.tile([128, Y1_S, Y1_S], BF16)
        y2 = bigpool.tile([128, Y2_S, Y2_S], BF16)
        acc = bigpool.tile([128, H * W], F32)
        bit = bigpool.tile([128, 3], F32)

        # --- setup ---
        nc.gpsimd.dma_start(out=bit, in_=b_in[:, :])
        # xo zero borders (both halves share the strip pattern; interiors
        # are overwritten by the two x DMAs below)
        P1 = PAD + 1
        nc.vector.memset(xo[:, 0:P1, :], 0.0)
        nc.vector.memset(xo[:, XO_S - 6:XO_S, :], 0.0)
        nc.vector.memset(xo[:, P1:XO_S - 6, 0:P1], 0.0)
        nc.vector.memset(xo[:, P1:XO_S - 6, XO_S - P1:XO_S], 0.0)
        # lower half: x at rows 5..100; upper half: x shifted up one row
        nc.sync.dma_start(out=xo[0:C, P1:P1 + H, P1:P1 + W], in_=x_in[:, :, :])
        nc.sync.dma_start(out=xo[C:128, P1 - 1:P1 - 1 + H, P1:P1 + W], in_=x_in[:, :, :])
        # y1/y2 zero borders for the even pairs (gpsimd; off critical path)
        nc.gpsimd.memset(y1[:, 0:4, :], 0.0)
        nc.gpsimd.memset(y1[:, Y1_S - 4:Y1_S, :], 0.0)
        nc.gpsimd.memset(y1[:, 4:Y1_S - 4, 0:4], 0.0)
        nc.gpsimd.memset(y1[:, 4:Y1_S - 4, Y1_S - 4:Y1_S], 0.0)
        nc.gpsimd.memset(y2[:, 0:2, :], 0.0)
        nc.gpsimd.memset(y2[:, Y2_S - 2:Y2_S, :], 0.0)
        nc.gpsimd.memset(y2[:, 2:Y2_S - 2, 0:2], 0.0)
        nc.gpsimd.memset(y2[:, 2:Y2_S - 2, Y2_S - 2:Y2_S], 0.0)

        bufs = [xo, y1, y2, None]

        for pk, (pa, pb) in enumerate(PAIRS):
            parity = pa % 2
            wt = wpool.tile([128, W_COLS], BF16, tag="wt")
            nc.gpsimd.dma_start(out=wt, in_=w_in[pk, :, :])

            for l in range(3):
                g = GEOM[(l, parity)]
                d, off, oh, ow, woff = g["d"], g["off"], g["oh"], g["ow"], g["woff"]
                src = bufs[l]
                dst = bufs[l + 1]
                nstream = 6 if l == 0 else 9
                slot0 = 0 if l == 0 else 6 + (l - 1) * 9
                h0 = 0
                while h0 < oh:
                    rr = min(R, oh - h0)
                    n = rr * ow
                    pt = psum.tile([128, 512], F32, tag="pt")
                    for j in range(nstream):
                        if l == 0:
                            kw = j % 3
                            kh0 = 0 if j < 3 else 2   # singles use kh=2
                            rbase = h0 + kh0 * d + off
                            cbase = kw * d + off
                        else:
                            kh, kw = divmod(j, 3)
                            rbase = h0 + kh * d + off
                            cbase = kw * d + off
                        lhsT = wt[:, (slot0 + j) * 128:(slot0 + j + 1) * 128]
                        rhs = src[0:128, rbase:rbase + rr, cbase:cbase + ow]
                        nc.tensor.matmul(pt[:, 0:n], lhsT, rhs,
                                         start=(j == 0), stop=(j == nstream - 1))
                    if l < 2:
                        a = h0 + woff
                        nc.scalar.activation(
                            out=dst[0:128, a:a + rr, woff:woff + ow],
                            in_=pt[:, 0:n].rearrange("p (r c) -> p r c", r=rr),
                            func=mybir.ActivationFunctionType.Relu,
                            bias=bit[:, l:l + 1], scale=1.0)
                    else:
                        # conv3: straight PSUM -> running channel max (fp32)
                        if pk == 0:
                            nc.vector.tensor_copy(acc[:, h0 * W:h0 * W + n],
                                                  pt[:, 0:n])
                        else:
                            nc.vector.tensor_max(acc[:, h0 * W:h0 * W + n],
                                                 acc[:, h0 * W:h0 * W + n],
                                                 pt[:, 0:n])
                    h0 += rr

        # --- tail: +b3, transpose blocks, channel max, relu+sigmoid+clip ---
        ident = bigpool.tile([128, 128], F32)
        masks.make_identity(nc, ident)
        red = bigpool.tile([128, 72], F32)
        NB = (H * W) // 128  # 72 blocks of 128 columns
        for gi in range(NB // 4):
            c0 = gi * 4 * 128
            nc.scalar.activation(
                out=acc[:, c0:c0 + 512], in_=acc[:, c0:c0 + 512],
                func=mybir.ActivationFunctionType.Identity,
                bias=bit[:, 2:3], scale=1.0)
            ps = tpsum.tile([128, 512], F32, tag="tp")
            for b in range(4):
                nc.tensor.transpose(ps[:, b * 128:(b + 1) * 128],
                                    acc[:, c0 + b * 128:c0 + (b + 1) * 128],
                                    ident)
            nc.vector.tensor_reduce(out=red[:, gi * 4:(gi + 1) * 4],
                                    in_=ps.rearrange("p (b c) -> p b c", b=4),
                                    axis=mybir.AxisListType.X,
                                    op=mybir.AluOpType.max)
        ps2 = tpsum.tile([128, 512], F32, tag="tp")
        nc.tensor.transpose(ps2[0:72, 0:128], red[:, :], ident)
        rsb = bigpool.tile([72, 128], F32)
        # relu (max with 0) then sigmoid then upper clip
        nc.scalar.activation(out=rsb, in_=ps2[0:72, 0:128],
                             func=mybir.ActivationFunctionType.Relu)
        nc.scalar.activation(out=rsb, in_=rsb,
                             func=mybir.ActivationFunctionType.Sigmoid)
        nc.vector.tensor_scalar(rsb, rsb, 1e-4, 1.0 - 1e-4,
                                mybir.AluOpType.max, mybir.AluOpType.min)
        nc.sync.dma_start(
            out=o_out.ap().rearrange("a (c r) -> a c r", r=128), in_=rsb)
    nc.compile()
    return nc


def _fold_weights(perms, dcn_w, dcn_b, conv2_w, conv2_b, conv3_w, conv3_b,
                  bn_gamma, bn_beta, bn_mean, bn_var):
    """Fold rotations/permutation/BN on the host.

    Returns (w_blob [4,128,W_COLS] bf16, biases [128,3] f32)."""
    scales = np.empty((3, C), np.float32)
    biases = np.empty((3, C), np.float32)
    conv_bs = [dcn_b, conv2_b, conv3_b]
    for l in range(3):
        s = bn_gamma[l] / np.sqrt(bn_var[l] + BN_EPS)
        scales[l] = s
        biases[l] = bn_beta[l] - bn_mean[l] * s + conv_bs[l] * s

    w_blob = np.zeros((4, 128, W_COLS), np.float32)
    base_ws = [dcn_w, conv2_w, conv3_w]
    for pk, pair in enumerate(PAIRS):
        for half, pi in enumerate(pair):
            m0 = half * C
            k = pi % 4
            for l in range(3):
                wl = np.rot90(base_ws[l], k=-k, axes=(-2, -1))
                if l == 0:
                    wl = np.einsum('omhw,mj->ojhw', wl, perms[pi], optimize=True)
                wl = wl * scales[l][:, None, None, None]  # fold BN scale
                if l == 0:
                    for j in range(6):
                        kw = j % 3
                        col = j * 128
                        if j < 3:
                            w_blob[pk, 0:C, col + m0:col + m0 + C] = wl[:, :, 0, kw].T
                            w_blob[pk, C:128, col + m0:col + m0 + C] = wl[:, :, 1, kw].T
                        else:
                            w_blob[pk, 0:C, col + m0:col + m0 + C] = wl[:, :, 2, kw].T
                else:
                    slot0 = 6 + (l - 1) * 9
                    for t in range(9):
                        kh, kw = divmod(t, 3)
                        col = (slot0 + t) * 128
                        # block-diagonal: K half == M half for each pass
                        w_blob[pk, m0:m0 + C, col + m0:col + m0 + C] = wl[:, :, kh, kw].T
    b128 = np.concatenate([biases.T, biases.T], axis=0)  # [128, 3]
    return w_blob.astype(ml_dtypes.bfloat16), np.ascontiguousarray(b128)


def kernel(x, perms, dcn_w, dcn_b, conv2_w, conv2_b, conv3_w, conv3_b,
           bn_gamma, bn_beta, bn_mean, bn_var):
    global LAST_EXEC_NS
    x = np.ascontiguousarray(np.asarray(x, np.float32))
    args = [np.asarray(a, np.float32) for a in
            (perms, dcn_w, dcn_b, conv2_w, conv2_b, conv3_w, conv3_b,
             bn_gamma, bn_beta, bn_mean, bn_var)]
    w_blob, b128 = _fold_weights(*args)
    x16 = x.astype(ml_dtypes.bfloat16)

    if "prog" not in _PROGRAM_CACHE:
        _PROGRAM_CACHE["prog"] = _build_program()
    nc = _PROGRAM_CACHE["prog"]

    in_maps = [{
        "x_in": np.ascontiguousarray(x16[b]),
        "w_in": w_blob,
        "b_in": b128,
    } for b in range(B)]

    r = run_bass_kernel_spmd(nc, in_maps, core_ids=list(range(B)), trace=TRACE)
    LAST_EXEC_NS = r.exec_time_ns
    out = np.stack([r.results[b]["o_out"].reshape(1, H, W) for b in range(B)])
    return out.astype(np.float32)
